# revision 1
# baseline (speedup 1.0000x reference)
"""DiceLoss kernel for Trainium2 (8 NeuronCores, SPMD data-parallel).

Problem: input [2,4,128,160,160] f32 logits, target [2,128,160,160] int
  pred = argmax(input, axis=1); for classes 1..3 compute
  inter_c = |pred==c & tgt==c|, union_c = |pred==c| + |tgt==c| - inter_c
  loss = 1 - mean_{b,c}( (inter+eps)/(union+eps) )

Sharding: flatten spatial dims (N=3,276,800 voxels per batch) and give each
of the 8 cores a contiguous 1/8 slice (S=409,600 voxels) of BOTH batches.
Each core computes per-(batch, class) partial counts; the host sums the 8
tiny partial-count tensors and finishes the scalar dice math.

Per-core on-chip layout: each (b, class) plane slice is [128 partitions x
3200]; processed in free-dim chunks of 1280/1280/640.

Engine assignment (per chunk):
  DVE : max01/max23/m (f32 max), pm1..3 = is_equal(x_c, m) -> bf16 masks
  ACT : tm_c = Relu(1 - Square(t - c)) -> bf16 one-hot of target, with
        accum_out giving the per-partition target counts for free
  PE  : inter_c via PM_c^T @ TM_c accumulated in PSUM (trace on host),
        pred counts via ones^T @ PM_c column sums
  DMA : PSUM -> DRAM drained directly (no SBUF staging)

(tensor_tensor_reduce and all GpSimd compute fail this toolchain's
walrus codegen — avoid.)

argmax tie semantics: pm_c = (x_c == max). For f32 normal inputs exact ties
have probability ~1e-7 over the whole tensor; the count error is <=O(1) out
of ~1e6, far below tolerance.
"""

import sys

sys.path.insert(0, "/opt/trn_rl_repo")

import numpy as np

# ---------------------------------------------------------------------------
# Hardcoded problem geometry
# ---------------------------------------------------------------------------
B = 2
C = 4
N_SP = 128 * 160 * 160        # 3,276,800 voxels per batch
N_CORES = 8
S = N_SP // N_CORES           # 409,600 voxels per core per batch
P = 128
SF = S // P                   # 3200 free elems per partition
# free-dim chunks (multiples of 128). Sizes ramp up so the first chunk's
# DMA+DVE latency (pipeline fill) is small.
CHUNKS = [(0, 256), (256, 512), (768, 1024), (1792, 1408)]
EPS = 1e-08

_CACHE = {}


def _build_bass(s=S, chunks=None):
    import concourse.bass as bass
    import concourse.tile as tile
    from concourse import bacc, mybir
    from contextlib import ExitStack

    if chunks is None:
        chunks = CHUNKS
    f32 = mybir.dt.float32
    bf16 = mybir.dt.bfloat16
    u8 = mybir.dt.uint8
    Alu = mybir.AluOpType

    # Bacc (not raw Bass): its compile() legalizes sync — multi-wait
    # instructions get their waits split onto event-semaphore nops, which
    # the walrus BIR verifier requires.
    nc = bacc.Bacc()

    n_chunks = len(chunks)
    n_cols = B * n_chunks * 3
    x = nc.declare_dram_parameter("x", [B, C, s], f32, isOutput=False)
    t = nc.declare_dram_parameter("t", [B, s], u8, isOutput=False)
    # acc_out[:, col] = tm_c counts (ACT accum), col = (b*n_chunks+j)*3+(c-1)
    acc_d = nc.declare_dram_parameter("acc_out", [P, n_cols], f32, isOutput=True)
    # diag_out[b][:, (c-1)*128 : c*128] = PM_c^T @ TM_c ; trace = inter_c
    diag_d = nc.declare_dram_parameter("diag_out", [B, P, 384], f32, isOutput=True)
    # cnt_out[b, 0, (c-1)*512:(c)*512] = per-column pm_c counts (PE colsum)
    cnt_d = nc.declare_dram_parameter("cnt_out", [B, 1, 1536], f32, isOutput=True)

    with ExitStack() as ctx:
        tc = ctx.enter_context(tile.TileContext(nc))
        const_pool = ctx.enter_context(tc.tile_pool(name="const", bufs=1))
        xpool = ctx.enter_context(tc.tile_pool(name="xp", bufs=3))
        tpool = ctx.enter_context(tc.tile_pool(name="tp", bufs=3))
        mpool = ctx.enter_context(tc.tile_pool(name="mp", bufs=2))
        kpool = ctx.enter_context(tc.tile_pool(name="kp", bufs=2))
        pspool = ctx.enter_context(tc.tile_pool(name="ps", bufs=1, space="PSUM"))

        acc = const_pool.tile([P, n_cols], f32)
        ones = const_pool.tile([P, 1], bf16)
        nc.vector.memset(ones[:], 1.0)
        # per-partition bias constants for activation (only 0.0/1.0 have
        # builtin const APs)
        neg_c = []
        for c in (1, 2, 3):
            bias_t = const_pool.tile([P, 1], f32, tag=f"bias{c}", name=f"bias{c}")
            nc.vector.memset(bias_t[:], -float(c))
            neg_c.append(bias_t)

        for b in range(B):
            ps_diags = [
                pspool.tile([P, 128], f32, tag=f"diag{ci}", name=f"ps_diag{ci}")
                for ci in range(3)
            ]
            ps_cnts = [
                pspool.tile([1, 512], f32, tag=f"cnt{ci}", name=f"ps_cnt{ci}")
                for ci in range(3)
            ]

            for j, (f0, F) in enumerate(chunks):
                xts = []
                for ci in range(C):
                    xc = xpool.tile([P, F], f32, tag=f"x{ci}", name=f"x{ci}")
                    xsrc = x[b, ci, :].rearrange("(p f) -> p f", p=P)
                    nc.sync.dma_start(out=xc[:], in_=xsrc[:, f0 : f0 + F])
                    xts.append(xc)
                tt = tpool.tile([P, F], u8, tag="tt")
                tsrc = t[b, :].rearrange("(p f) -> p f", p=P)
                nc.sync.dma_start(out=tt[:], in_=tsrc[:, f0 : f0 + F])

                m01 = mpool.tile([P, F], f32, tag="m01")
                nc.vector.tensor_tensor(m01[:], xts[0][:], xts[1][:], op=Alu.max)
                m23 = mpool.tile([P, F], f32, tag="m23")
                nc.vector.tensor_tensor(m23[:], xts[2][:], xts[3][:], op=Alu.max)
                m = mpool.tile([P, F], f32, tag="m")
                nc.vector.tensor_tensor(m[:], m01[:], m23[:], op=Alu.max)

                pms = []
                for ci in range(3):
                    pm = kpool.tile([P, F], bf16, tag=f"pm{ci}", name=f"pm{ci}")
                    nc.vector.tensor_tensor(
                        pm[:], xts[ci + 1][:], m[:], op=Alu.is_equal
                    )
                    pms.append(pm)

                tms = []
                for ci, c in enumerate((1, 2, 3)):
                    sq = kpool.tile([P, F], bf16, tag=f"sq{c}", name=f"sq{c}")
                    nc.scalar.activation(
                        sq[:], tt[:], mybir.ActivationFunctionType.Square,
                        bias=neg_c[ci][:], scale=1.0,
                    )
                    tm = kpool.tile([P, F], bf16, tag=f"tm{c}", name=f"tm{c}")
                    col = (b * n_chunks + j) * 3 + ci
                    nc.scalar.activation(
                        tm[:], sq[:], mybir.ActivationFunctionType.Relu,
                        bias=1.0, scale=-1.0,
                        accum_out=acc[:, col : col + 1],
                    )
                    tms.append(tm)

                last_j = j == n_chunks - 1
                # inter_c: PM_c^T @ TM_c accumulated over the whole batch
                ns128 = F // 128
                for si in range(ns128):
                    sl = slice(si * 128, (si + 1) * 128)
                    for ci in range(3):
                        nc.tensor.matmul(
                            ps_diags[ci][:, :],
                            pms[ci][:, sl],
                            tms[ci][:, sl],
                            start=(j == 0 and si == 0),
                            stop=(last_j and si == ns128 - 1),
                        )
                # pm_c counts: ones^T @ PM_c partition-sums, accumulated
                offs = []
                off = 0
                while off < F:
                    offs.append((off, min(512, F - off)))
                    off += 512
                for ci in range(3):
                    for oi, (off, ns) in enumerate(offs):
                        nc.tensor.matmul(
                            ps_cnts[ci][0:1, 0:ns],
                            ones[:],
                            pms[ci][:, off : off + ns],
                            start=(j == 0 and oi == 0),
                            stop=(last_j and oi == len(offs) - 1),
                        )

            # drain PSUM -> SBUF (DMA cannot read PSUM), then DMA out
            cw = min(512, max(F for _, F in chunks))
            sb_diag = tpool.tile([P, 384], f32, tag="sbd", name="sb_diag")
            for ci in range(3):
                nc.scalar.copy(
                    sb_diag[:, ci * 128 : (ci + 1) * 128], ps_diags[ci][:]
                )
            nc.sync.dma_start(out=diag_d[b, :, :], in_=sb_diag[:])
            sb_cnt = tpool.tile([1, 1536], f32, tag="sbc", name="sb_cnt")
            for ci in range(3):
                nc.vector.tensor_copy(
                    sb_cnt[0:1, ci * 512 : ci * 512 + cw], ps_cnts[ci][0:1, 0:cw]
                )
                # unwritten columns of cnt_d stay zero (outputs are
                # zero-initialized by the runtime)
                nc.sync.dma_start(
                    out=cnt_d[b, :, ci * 512 : ci * 512 + cw],
                    in_=sb_cnt[0:1, ci * 512 : ci * 512 + cw],
                )

        nc.sync.dma_start(out=acc_d[:, :], in_=acc[:])

    nc.compile()
    return nc


def _get_nc():
    if "nc" not in _CACHE:
        _CACHE["nc"] = _build_bass()
    return _CACHE["nc"]


def _shard_inputs(input, target):
    inp = np.ascontiguousarray(input, dtype=np.float32).reshape(B, C, N_SP)
    tgt = np.asarray(target).reshape(B, N_SP)
    in_maps = []
    for r in range(N_CORES):
        xr = np.ascontiguousarray(inp[:, :, r * S : (r + 1) * S])
        tr = np.ascontiguousarray(tgt[:, r * S : (r + 1) * S].astype(np.uint8))
        in_maps.append({"x": xr, "t": tr})
    return in_maps


def _finish(results):
    """Combine per-core partial counts into the dice loss."""
    inter = np.zeros((B, 3), np.float64)
    pred_cnt = np.zeros((B, 3), np.float64)
    tgt_cnt = np.zeros((B, 3), np.float64)
    n_chunks = len(CHUNKS)
    for res in results:
        acc = np.asarray(res["acc_out"], np.float64)        # [128, n_cols]
        diag = np.asarray(res["diag_out"], np.float64)      # [B, 128, 384]
        cnt = np.asarray(res["cnt_out"], np.float64)        # [B, 1, 1536]
        for b in range(B):
            for ci in range(3):
                blk = diag[b][:, ci * 128 : (ci + 1) * 128]
                inter[b, ci] += np.trace(blk)
                pred_cnt[b, ci] += cnt[b, 0, ci * 512 : (ci + 1) * 512].sum()
                for j in range(n_chunks):
                    tgt_cnt[b, ci] += acc[:, (b * n_chunks + j) * 3 + ci].sum()
    union = pred_cnt + tgt_cnt - inter
    dice = (inter + EPS) / (union + EPS)
    return np.float32(1.0 - dice.mean())


def kernel(input, target):
    from concourse.bass_utils import run_bass_kernel_spmd

    nc = _get_nc()
    in_maps = _shard_inputs(input, target)
    out = run_bass_kernel_spmd(nc, in_maps, core_ids=list(range(N_CORES)))
    return _finish(out.results)


if __name__ == "__main__":
    # Smoke test with random data against a numpy reference.
    rng = np.random.default_rng(0)
    inp = rng.standard_normal((B, C, 128, 160, 160), dtype=np.float32)
    tgt = rng.integers(0, C, size=(B, 128, 160, 160)).astype(np.int32)

    got = kernel(input=inp, target=tgt)

    pred = np.argmax(inp, axis=1).reshape(B, -1)
    tg = tgt.reshape(B, -1)
    dice = np.zeros((B, 3))
    for b in range(B):
        for ci, c in enumerate((1, 2, 3)):
            pm = pred[b] == c
            tm = tg[b] == c
            i = np.sum(pm & tm)
            u = np.sum(pm | tm)
            dice[b, ci] = (i + EPS) / (u + EPS)
    want = np.float32(1.0 - dice.mean())
    print("kernel:", got, "reference:", want, "relerr:", abs(got - want) / abs(want))



# revision 9
# speedup vs baseline: 18.6213x; 18.6213x over previous
"""DiceLoss kernel for Trainium2 (8 NeuronCores, SPMD data-parallel).

Problem: input [2,4,128,160,160] f32 logits, target [2,128,160,160] int
  pred = argmax(input, axis=1); for classes 1..3 compute
  inter_c = |pred==c & tgt==c|, union_c = |pred==c| + |tgt==c| - inter_c
  loss = 1 - mean_{b,c}( (inter+eps)/(union+eps) )

Sharding: flatten spatial dims (N=3,276,800 voxels per batch) and give each
of the 8 cores a contiguous 1/8 slice (S=409,600 voxels) of BOTH batches.
Each core computes per-(batch, class) partial counts; the host sums the 8
tiny partial-count tensors and finishes the scalar dice math.

Per-core on-chip layout: each (b, class) plane slice is [128 partitions x
3200]; processed in free-dim chunks of 384/1280/1536 (small first chunk to
shorten pipeline fill; few chunks to amortize per-instruction overheads:
DVE tensor_tensor costs ~(F+151) cycles @0.96GHz, ACT ~(F+352) @1.2GHz).

Engine assignment (per chunk):
  DVE : max01/max23/m (f32 max), pm_c = is_equal(x_c, m) -> bf16 masks
        (6 f32 1x passes - the engine floor for this algorithm)
  ACT : tm_c = Relu(1 - Square(t - c)) -> bf16 one-hot of target, written
        in 129-strided groups (see below), accum_out giving per-partition
        target counts for free; also drains PSUM (keeps DVE free)
  PE  : one [128x129] matmul per 128-slice per class: stationary
        pm_c[:, sl], moving [tm_c slice | ones column]. Cols 0..127
        accumulate PM_c^T @ TM_c trace blocks (host takes the trace =
        inter_c); col 128 accumulates per-stationary-col pm sums (host
        sums = pred count). No separate column-sum matmuls needed.
  GpSimd: memsets of the interleaved ones columns
  DMA : all loads/stores issued from Sync (HWDGE); x tiles triple-buffered

tm tile layout: [128, (F/128)*129]; group g holds 128 one-hot cols at
[129g : 129g+128] and a constant 1.0 column at 129g+128, so the matmul
moving operand tile[:, 129g : 129g+129] is contiguous.

(tensor_tensor_reduce and all GpSimd compute fail this toolchain's
walrus codegen - avoid. GpSimd memset is fine - the baseline used it.)

argmax tie semantics: pm_c = (x_c == m). For f32 normal inputs exact ties
have probability ~1e-7 over the whole tensor; the count error is <=O(1) out
of ~1e6, far below tolerance.
"""

import sys

sys.path.insert(0, "/opt/trn_rl_repo")

import numpy as np

# ---------------------------------------------------------------------------
# Hardcoded problem geometry
# ---------------------------------------------------------------------------
B = 2
C = 4
N_SP = 128 * 160 * 160        # 3,276,800 voxels per batch
N_CORES = 8
S = N_SP // N_CORES           # 409,600 voxels per core per batch
P = 128
SF = S // P                   # 3200 free elems per partition
# free-dim chunks (multiples of 128). Small first chunk shortens the
# pipeline fill; 3 chunks total keeps per-instruction overhead low.
CHUNKS = [(0, 384), (384, 1280), (1664, 1536)]
EPS = 1e-08

_CACHE = {}


def _build_bass(s=S, chunks=None):
    import concourse.bass as bass
    import concourse.tile as tile
    from concourse import bacc, mybir
    from contextlib import ExitStack

    if chunks is None:
        chunks = CHUNKS
    f32 = mybir.dt.float32
    bf16 = mybir.dt.bfloat16
    u8 = mybir.dt.uint8
    Alu = mybir.AluOpType

    # Bacc (not raw Bass): its compile() legalizes sync — multi-wait
    # instructions get their waits split onto event-semaphore nops, which
    # the walrus BIR verifier requires.
    nc = bacc.Bacc()

    n_chunks = len(chunks)
    n_cols = B * n_chunks * 3
    x = nc.declare_dram_parameter("x", [B, C, s], f32, isOutput=False)
    t = nc.declare_dram_parameter("t", [B, s], u8, isOutput=False)
    # acc_out[:, col] = tm_c counts (ACT accum), col = (b*n_chunks+j)*3+(c-1)
    acc_d = nc.declare_dram_parameter("acc_out", [P, n_cols], f32, isOutput=True)
    # diag_out[b][:, 129*(c-1) : 129*c]: cols 0..127 = PM_c^T @ TM_c block
    # (trace = inter_c), col 128 = per-col pm sums (sum = pred count)
    diag_d = nc.declare_dram_parameter("diag_out", [B, P, 387], f32, isOutput=True)

    with ExitStack() as ctx:
        tc = ctx.enter_context(tile.TileContext(nc))
        const_pool = ctx.enter_context(tc.tile_pool(name="const", bufs=1))
        xpool = ctx.enter_context(tc.tile_pool(name="xp", bufs=3))
        tpool = ctx.enter_context(tc.tile_pool(name="tp", bufs=3))
        mpool = ctx.enter_context(tc.tile_pool(name="mp", bufs=1))
        kpool = ctx.enter_context(tc.tile_pool(name="kp", bufs=2))
        opool = ctx.enter_context(tc.tile_pool(name="op", bufs=1))
        pspool = ctx.enter_context(tc.tile_pool(name="ps", bufs=1, space="PSUM"))

        acc = const_pool.tile([P, n_cols], f32)
        # per-partition bias constants for activation (only 0.0/1.0 have
        # builtin const APs)
        neg_c = []
        for c in (1, 2, 3):
            bias_t = const_pool.tile([P, 1], f32, tag=f"bias{c}", name=f"bias{c}")
            nc.vector.memset(bias_t[:], -float(c))
            neg_c.append(bias_t)

        for b in range(B):
            # one PSUM tile per class: matmul start=True clears has_written
            # at bank granularity, so accumulation groups must not share a
            # bank with other groups that start later
            ps_diags = [
                pspool.tile([P, 129], f32, tag=f"diag{ci}", name=f"ps_diag{ci}")
                for ci in range(3)
            ]

            for j, (f0, F) in enumerate(chunks):
                ng = F // 128  # number of 128-col groups in this chunk
                xts = []
                for ci in range(C):
                    xc = xpool.tile([P, F], f32, tag=f"x{ci}", name=f"x{ci}")
                    xsrc = x[b, ci, :].rearrange("(p f) -> p f", p=P)
                    nc.sync.dma_start(out=xc[:], in_=xsrc[:, f0 : f0 + F])
                    xts.append(xc)
                tt = tpool.tile([P, F], u8, tag="tt")
                tsrc = t[b, :].rearrange("(p f) -> p f", p=P)
                nc.sync.dma_start(out=tt[:], in_=tsrc[:, f0 : f0 + F])

                m01 = mpool.tile([P, F], f32, tag="m01")
                nc.vector.tensor_tensor(m01[:], xts[0][:], xts[1][:], op=Alu.max)
                m23 = mpool.tile([P, F], f32, tag="m23")
                nc.vector.tensor_tensor(m23[:], xts[2][:], xts[3][:], op=Alu.max)
                m = mpool.tile([P, F], f32, tag="m")
                nc.vector.tensor_tensor(m[:], m01[:], m23[:], op=Alu.max)

                pms = []
                for ci in range(3):
                    pm = kpool.tile([P, F], bf16, tag=f"pm{ci}", name=f"pm{ci}")
                    nc.vector.tensor_tensor(
                        pm[:], xts[ci + 1][:], m[:], op=Alu.is_equal
                    )
                    pms.append(pm)

                tms = []
                for ci, c in enumerate((1, 2, 3)):
                    sq = kpool.tile([P, F], bf16, tag=f"sq{c}", name=f"sq{c}")
                    nc.scalar.activation(
                        sq[:], tt[:], mybir.ActivationFunctionType.Square,
                        bias=neg_c[ci][:], scale=1.0,
                    )
                    # tm tile with an interleaved ones column per 128-group:
                    # [128 one-hot cols | 1.0] x ng, so each matmul moving
                    # operand is a contiguous 129-col window
                    tm = kpool.tile(
                        [P, ng, 129], bf16, tag=f"tm{c}", name=f"tm{c}"
                    )
                    nc.gpsimd.memset(tm[:, :, 128:129], 1.0)
                    col = (b * n_chunks + j) * 3 + ci
                    nc.scalar.activation(
                        tm[:, :, 0:128], sq[:].rearrange("p (g q) -> p g q", q=128),
                        mybir.ActivationFunctionType.Relu,
                        bias=1.0, scale=-1.0,
                        accum_out=acc[:, col : col + 1],
                    )
                    tms.append(tm)

                last_j = j == n_chunks - 1
                # inter_c + pred counts in one [128x129] matmul per slice,
                # accumulated over the whole batch
                for si in range(ng):
                    sl = slice(si * 128, (si + 1) * 128)
                    for ci in range(3):
                        nc.tensor.matmul(
                            ps_diags[ci][:, :],
                            pms[ci][:, sl],
                            tms[ci][:, si, :],
                            start=(j == 0 and si == 0),
                            stop=(last_j and si == ng - 1),
                        )

            # drain PSUM -> SBUF on ACT (keeps DVE free), then DMA out
            sb_diag = opool.tile([P, 387], f32, tag="sbd", name="sb_diag")
            for ci in range(3):
                nc.scalar.copy(
                    sb_diag[:, ci * 129 : (ci + 1) * 129], ps_diags[ci][:, :]
                )
            nc.sync.dma_start(out=diag_d[b, :, :], in_=sb_diag[:])

        nc.sync.dma_start(out=acc_d[:, :], in_=acc[:])

    nc.compile()
    return nc


def _get_nc():
    if "nc" not in _CACHE:
        _CACHE["nc"] = _build_bass()
    return _CACHE["nc"]


def _shard_inputs(input, target):
    inp = np.ascontiguousarray(input, dtype=np.float32).reshape(B, C, N_SP)
    tgt = np.asarray(target).reshape(B, N_SP)
    in_maps = []
    for r in range(N_CORES):
        xr = np.ascontiguousarray(inp[:, :, r * S : (r + 1) * S])
        tr = np.ascontiguousarray(tgt[:, r * S : (r + 1) * S].astype(np.uint8))
        in_maps.append({"x": xr, "t": tr})
    return in_maps


def _finish(results):
    """Combine per-core partial counts into the dice loss."""
    inter = np.zeros((B, 3), np.float64)
    pred_cnt = np.zeros((B, 3), np.float64)
    tgt_cnt = np.zeros((B, 3), np.float64)
    n_chunks = len(CHUNKS)
    for res in results:
        acc = np.asarray(res["acc_out"], np.float64)        # [128, n_cols]
        diag = np.asarray(res["diag_out"], np.float64)      # [B, 128, 387]
        for b in range(B):
            for ci in range(3):
                blk = diag[b][:, ci * 129 : (ci + 1) * 129]
                inter[b, ci] += np.trace(blk[:, 0:128])
                pred_cnt[b, ci] += blk[:, 128].sum()
                for j in range(n_chunks):
                    tgt_cnt[b, ci] += acc[:, (b * n_chunks + j) * 3 + ci].sum()
    union = pred_cnt + tgt_cnt - inter
    dice = (inter + EPS) / (union + EPS)
    return np.float32(1.0 - dice.mean())


def kernel(input, target):
    from concourse.bass_utils import run_bass_kernel_spmd

    nc = _get_nc()
    in_maps = _shard_inputs(input, target)
    out = run_bass_kernel_spmd(nc, in_maps, core_ids=list(range(N_CORES)))
    return _finish(out.results)


if __name__ == "__main__":
    # Smoke test with random data against a numpy reference.
    rng = np.random.default_rng(0)
    inp = rng.standard_normal((B, C, 128, 160, 160), dtype=np.float32)
    tgt = rng.integers(0, C, size=(B, 128, 160, 160)).astype(np.int32)

    got = kernel(input=inp, target=tgt)

    pred = np.argmax(inp, axis=1).reshape(B, -1)
    tg = tgt.reshape(B, -1)
    dice = np.zeros((B, 3))
    for b in range(B):
        for ci, c in enumerate((1, 2, 3)):
            pm = pred[b] == c
            tm = tg[b] == c
            i = np.sum(pm & tm)
            u = np.sum(pm | tm)
            dice[b, ci] = (i + EPS) / (u + EPS)
    want = np.float32(1.0 - dice.mean())
    print("kernel:", got, "reference:", want, "relerr:", abs(got - want) / abs(want))
